# revision 9
# baseline (speedup 1.0000x reference)
"""Trainium2 Bass kernel for ExtractRelevantPatches (pool -> top-k -> gather).

Full-input contract: kernel(heatmap [64,448,448,1] f32, image [64,448,448,3] f32)
-> [1344, 64, 64, 3] f32.

Sharding: pure data-parallel over batch; 8 batches per NeuronCore, 8 cores.

Pipelined per-core algorithm (raw Bass, explicit semaphores), 4 blocks of
2 batches each, so gathers/stores of early blocks overlap pooling/top-k of
later blocks:

  Per block g (batches 2g, 2g+1):
  1. L_g  (sync HWDGE): heatmap block -> SBUF [128, 7, 448] with partition
     p = 64*bp + (r % 64)  (bp = batch parity in block), free = (r//64, col).
  2. R_g  (DVE): reduce_sum over 64-col groups -> red_g [128, 7, 7].
  3. P_g  (TensorE): matmul with 0/1 matrix G64 [128,2] (G64[p,b]=1 iff
     p//64==b) -> psV_g [2, 49] = per-batch pooled sums, batch on partition.
     Ranking by sums == ranking by means.  No cross-partition shuffle needed.
  4. T_g  (DVE): top-24 via 3 rounds of max + max_index + match_replace;
     keep first 21 (descending, as jax top_k).  base = idx + 441*(idx//7)
     (patch-row units of 192 elems) -> base_f_g [2, 21] f32.
  5. F_g  (scalar HWDGE): SBUF->SBUF DMA flatten [2,21] -> basef_g [1,42].
  6. B_g  (TensorE): ones[1,128] matmul broadcast, x4 along free ->
     psD_g [128, 168] (all partitions identical).
  7. X_g  (DVE): idx16 block slice = cast_i16(psD_g) + static table
     (112*(s%4) + 7*w + 3136*b terms), 16-wrapped, replicated x8 cores.
  8. G_g  (gpsimd SWDGE): 3x dma_gather of 896 patch-rows (768B each)
     DRAM->SBUF GT[:, 21g:21g+21, :].  A dummy 128-idx gather at kernel
     start pre-warms the ucode path.
  9. S_g  (sync HWDGE): store GT cols -> DRAM out rows [2688g, 2688(g+1)).
"""

import numpy as np

_N_CORES = 8
_B = 64
_B_LOC = _B // _N_CORES  # 8
_PATCH = 64
_GRID = 7
_NPATCH = 21
_PROW = _PATCH * 3            # 192 elements per patch-row
_OUT_ROWS_LOC = _B_LOC * _NPATCH  # 168
_NBLK = 4                     # blocks of 2 batches

_nc_cache = None


def build_program():
    """Build the per-core SPMD Bass program (cached)."""
    global _nc_cache
    if _nc_cache is not None:
        return _nc_cache

    import concourse.bass as bass
    import concourse.bacc as bacc
    import concourse.mybir as mybir

    f32 = mybir.dt.float32
    i16 = mybir.dt.int16
    i32 = mybir.dt.int32
    u32 = mybir.dt.uint32
    X = mybir.AxisListType.X
    Op = mybir.AluOpType

    nc = bacc.Bacc(num_swdge_queues=4)

    hm_in = nc.declare_dram_parameter(
        "heatmap", [_B_LOC, 448, 448, 1], f32, isOutput=False)
    img_in = nc.declare_dram_parameter(
        "image", [_B_LOC, 448, 448, 3], f32, isOutput=False)
    out_t = nc.declare_dram_parameter(
        "out", [_OUT_ROWS_LOC, _PATCH, _PATCH, 3], f32, isOutput=True)

    # Static part of the gather index list, int16 [16, 672]:
    # position i = R sits at [R%16, R//16]; R = 16*s + w;
    # static term = 7*(R%64) + 3136*(R//1344) = 112*(s%4) + 7*w + 3136*(s//84)
    s_ar = np.arange(672, dtype=np.int64)
    w_ar = np.arange(16, dtype=np.int64)
    st = (112 * (s_ar[None, :] % 4) + 7 * w_ar[:, None]
          + 3136 * (s_ar[None, :] // 84)).astype(np.int16)
    st = np.tile(st, (8, 1))  # replicate across the 8 gpsimd cores
    sttab_const = nc.inline_tensor(st, name="sttab_const")

    # heatmap batch view [8, 64, 7, 448]: partition rl = r%64, free (r//64, c)
    hm_bat = (hm_in[:]
              .rearrange("b (rhi rl) c one -> b rl rhi (c one)", rl=64))

    # image patch-row view: [25088, 192]
    img_rows = (img_in[:]
                .rearrange("b r c ch -> (b r c ch)")
                .rearrange("(n e) -> n e", e=_PROW))

    # output patch-row view [10752, 192] -> [p, c, e] with R = 128*c + p
    out_pc = (out_t[:]
              .rearrange("r a b c -> (r a b c)")
              .rearrange("(n e) -> n e", e=_PROW)
              .rearrange("(c p) e -> p c e", p=128))

    from contextlib import ExitStack

    with ExitStack() as ctx:
        e = ctx.enter_context
        hmB = [e(nc.sbuf_tensor(f"hm{g}", [128, 7, 448], f32))
               for g in range(_NBLK)]
        red = [e(nc.sbuf_tensor(f"red{g}", [128, 7, 7], f32))
               for g in range(_NBLK)]
        G64 = e(nc.sbuf_tensor("G64", [128, 2], f32))
        ones128 = e(nc.sbuf_tensor("ones128", [1, 128], f32))
        Vt = e(nc.sbuf_tensor("Vt", [2, 49], f32))
        vw0 = e(nc.sbuf_tensor("vw0", [2, 49], f32))
        vw1 = e(nc.sbuf_tensor("vw1", [2, 49], f32))
        m2 = e(nc.sbuf_tensor("m2", [2, 8], f32))
        idxu = e(nc.sbuf_tensor("idxu", [2, 24], u32))
        t1 = e(nc.sbuf_tensor("t1", [2, _NPATCH], i32))
        t2 = e(nc.sbuf_tensor("t2", [2, _NPATCH], i32))
        base_f = [e(nc.sbuf_tensor(f"base_f{g}", [2, _NPATCH], f32))
                  for g in range(_NBLK)]
        basef = [e(nc.sbuf_tensor(f"basef{g}", [1, 42], f32))
                 for g in range(_NBLK)]
        sttab = e(nc.sbuf_tensor("sttab", [128, 672], i16))
        idx16 = e(nc.sbuf_tensor("idx16", [128, 672], i16))
        warmidx = e(nc.sbuf_tensor("warmidx", [128, 8], i16))
        GT = e(nc.sbuf_tensor("GT", [128, 84, _PROW], f32))
        GTwarm = e(nc.sbuf_tensor("GTwarm", [128, 1, _PROW], f32))
        psV = [e(nc.psum_tensor(f"psV{g}", [2, 49], f32))
               for g in range(_NBLK)]
        psD = [e(nc.psum_tensor(f"psD{g}", [128, 168], f32))
               for g in range(_NBLK)]
        s_load = e(nc.semaphore("s_load"))
        s_stt = e(nc.semaphore("s_stt"))
        s_red = e(nc.semaphore("s_red"))
        s_pmm = e(nc.semaphore("s_pmm"))
        s_topk = e(nc.semaphore("s_topk"))
        s_flat = e(nc.semaphore("s_flat"))
        s_bmm = e(nc.semaphore("s_bmm"))
        s_idx = e(nc.semaphore("s_idx"))
        s_ones = e(nc.semaphore("s_ones"))
        s_warm = e(nc.semaphore("s_warm"))
        s_g = [e(nc.semaphore(f"s_g{g}")) for g in range(_NBLK)]
        s_st = e(nc.semaphore("s_st"))
        block = e(nc.Block())

        @block.sync
        def _(sync):
            for g in range(_NBLK):
                for bp in range(2):
                    sync.dma_start(
                        out=hmB[g][64 * bp:64 * bp + 64, :, :],
                        in_=hm_bat[2 * g + bp],
                    ).then_inc(s_load, 16)
            for g in range(_NBLK):
                sync.wait_ge(s_g[g], 48)
                sync.dma_start(
                    out=out_pc[:, 21 * g:21 * g + 21, :],
                    in_=GT[:, 21 * g:21 * g + 21, :],
                ).then_inc(s_st, 16)
            sync.wait_ge(s_st, 64)

        @block.scalar
        def _(sc):
            sc.dma_start(out=sttab[:], in_=sttab_const[:]).then_inc(s_stt, 16)
            for g in range(_NBLK):
                # flatten [2,21] -> [1,42] across partitions (SBUF->SBUF)
                sc.wait_ge(s_topk, g + 1)
                sc.dma_start(
                    out=basef[g][:],
                    in_=base_f[g][:],
                ).then_inc(s_flat, 16)

        @block.vector
        def _(vector):
            # constants (disjoint writes, no deps)
            vector.memset(G64[0:64, 0:1], 1.0)
            vector.memset(G64[0:64, 1:2], 0.0)
            vector.memset(G64[64:128, 0:1], 0.0)
            vector.memset(G64[64:128, 1:2], 1.0)
            vector.memset(ones128[:], 1.0)
            vector.memset(warmidx[:], 0)
            vector.drain().then_inc(s_ones, 1)

            def R_stage(g):
                vector.wait_ge(s_load, 32 * (g + 1))
                vector.reduce_sum(
                    out=red[g][:],
                    in_=hmB[g][:].rearrange("p rhi (bc u) -> p rhi bc u",
                                            u=64),
                    axis=X,
                )
                vector.drain().then_inc(s_red, 1)

            def T_stage(g):
                vector.wait_ge(s_pmm, g + 1)
                vector.tensor_copy(out=Vt[:], in_=psV[g][:])
                vector.drain()
                cur = Vt
                for r3 in range(3):
                    vector.max(out=m2[:], in_=cur[:])
                    vector.drain()
                    vector.max_index(
                        out=idxu[:, 8 * r3:8 * r3 + 8], in_max=m2[:],
                        in_values=cur[:])
                    if r3 < 2:
                        nxt = vw0 if r3 == 0 else vw1
                        vector.match_replace(
                            out=nxt[:], in_to_replace=m2[:], in_values=cur[:],
                            imm_value=-1e30)
                        vector.drain()
                        cur = nxt
                vector.drain()
                # base = idx + 441*(idx//7); idx//7 via (idx*9363)>>16
                idx_i = idxu[:, :_NPATCH].bitcast(i32)
                vector.tensor_scalar(
                    out=t1[:], in0=idx_i, scalar1=9363, scalar2=None,
                    op0=Op.mult)
                vector.drain()
                vector.tensor_scalar(
                    out=t1[:], in0=t1[:], scalar1=16, scalar2=None,
                    op0=Op.logical_shift_right)
                vector.drain()
                vector.scalar_tensor_tensor(
                    out=t2[:], in0=t1[:], scalar=441, in1=idx_i,
                    op0=Op.mult, op1=Op.add)
                vector.drain()
                vector.tensor_copy(out=base_f[g][:], in_=t2[:])
                vector.drain().then_inc(s_topk, 1)

            def X_stage(g):
                vector.wait_ge(s_bmm, g + 1)
                if g == 0:
                    vector.wait_ge(s_stt, 16)
                sl = slice(168 * g, 168 * g + 168)
                vector.tensor_copy(out=idx16[:, sl], in_=psD[g][:])
                vector.drain()
                vector.tensor_tensor(
                    out=idx16[:, sl], in0=idx16[:, sl], in1=sttab[:, sl],
                    op=Op.add)
                vector.drain().then_inc(s_idx, 1)

            # interleave: X_g right after R_{g+1} so the idx of block g is
            # ready as soon as its broadcast matmul lands, without letting
            # the next top-k delay it
            R_stage(0)
            T_stage(0)
            R_stage(1)
            X_stage(0)
            T_stage(1)
            R_stage(2)
            X_stage(1)
            T_stage(2)
            R_stage(3)
            X_stage(2)
            T_stage(3)
            X_stage(3)

        @block.tensor
        def _(tensor):
            def P_stage(g):
                tensor.wait_ge(s_red, g + 1)
                if g == 0:
                    tensor.wait_ge(s_ones, 1)
                tensor.matmul(
                    out=psV[g][:],
                    lhsT=G64[:],
                    rhs=red[g][:].rearrange("p rhi bc -> p (rhi bc)"),
                    start=True,
                    stop=True,
                ).then_inc(s_pmm, 1)

            def B_stage(g):
                tensor.wait_ge(s_flat, 16 * (g + 1))
                bb = basef[g][:1, :].rearrange("p (m one) -> p m one", one=1)
                tensor.matmul(
                    out=psD[g][:],
                    lhsT=ones128[:],
                    rhs=bb[:, 0:42, :].to_broadcast([1, 42, 4]),
                    start=True, stop=True,
                ).then_inc(s_bmm, 1)

            P_stage(0)
            P_stage(1)
            B_stage(0)
            P_stage(2)
            B_stage(1)
            P_stage(3)
            B_stage(2)
            B_stage(3)

        @block.gpsimd
        def _(g):
            # preload the extended-instruction library early so the ucode
            # overlay DMA overlaps the heatmap phase
            from concourse import library_config
            g.load_library(library_config.mlp)
            # dummy gather to absorb the cold-start cost of the gather ucode
            g.wait_ge(s_ones, 1)
            g.dma_gather(
                out_ap=GTwarm[:],
                in_ap=img_rows,
                idxs_ap=warmidx[:],
                num_idxs=128,
                num_idxs_reg=128,
                elem_size=_PROW,
                queue_num=0,
            ).then_inc(s_warm, 16)
            g.wait_ge(s_warm, 16)
            # real gathers: 3 calls of 896 patch-rows per block
            for blk in range(_NBLK):
                g.wait_ge(s_idx, blk + 1)
                for c in range(3):
                    k = 3 * blk + c
                    g.dma_gather(
                        out_ap=GT[:, 7 * k:7 * k + 7, :],
                        in_ap=img_rows,
                        idxs_ap=idx16[:, 56 * k:56 * k + 56],
                        num_idxs=896,
                        num_idxs_reg=896,
                        elem_size=_PROW,
                        queue_num=k % 4,
                    ).then_inc(s_g[blk], 16)

    nc.finalize()
    _nc_cache = nc
    return nc


def kernel(heatmap, image):
    from concourse.bass_utils import run_bass_kernel_spmd

    heatmap = np.ascontiguousarray(np.asarray(heatmap), dtype=np.float32)
    image = np.ascontiguousarray(np.asarray(image), dtype=np.float32)
    assert heatmap.shape == (_B, 448, 448, 1)
    assert image.shape == (_B, 448, 448, 3)

    nc = build_program()
    in_maps = [
        {
            "heatmap": heatmap[c * _B_LOC:(c + 1) * _B_LOC],
            "image": image[c * _B_LOC:(c + 1) * _B_LOC],
        }
        for c in range(_N_CORES)
    ]
    res = run_bass_kernel_spmd(nc, in_maps, list(range(_N_CORES)))
    outs = [res.results[c]["out"] for c in range(_N_CORES)]
    return np.concatenate(outs, axis=0)


# revision 18
# speedup vs baseline: 1.1365x; 1.1365x over previous
"""Trainium2 Bass kernel for ExtractRelevantPatches (pool -> top-k -> gather).

Full-input contract: kernel(heatmap [64,448,448,1] f32, image [64,448,448,3] f32)
-> [1344, 64, 64, 3] f32.

Sharding: pure data-parallel over batch; 8 batches per NeuronCore, 8 cores.

Pipelined per-core algorithm (raw Bass, explicit semaphores), 4 blocks of
2 batches, so gather descriptor expansion + HBM flow of early blocks overlap
pool/top-k of later blocks:

  Per block g (batches 2g, 2g+1):
  1. L_g: heatmap batch 2g on the sync HWDGE queue -> SBUF partitions 0-63,
     batch 2g+1 on the scalar HWDGE queue -> partitions 64-127 (two queues
     so both 64-partition streams run concurrently; SBUF write bw is
     ~2.9 GB/s/partition).  Tile [128, 7, 448], partition = 64*bp + r%64.
  2. R_g (DVE): reduce_sum over 64-col groups -> red_g [128, 7, 7].
  3. P_g (TensorE): matmul with 0/1 matrix G64 (G64[p,b]=1 iff p//64==b)
     -> psV_g [2, 49]: per-batch pooled sums on their own partition.
  4. T_g (DVE): top-24 via 3 rounds of max8/find_index8/match_replace8,
     keep first 21 (descending, matching jax top_k); cast idx -> f32.
  5. A_g (Act): u = (idx + 0.5) * (1/7)  (scale+bias in one activation).
  6. C_g (DVE): br = cast_i32(u) (truncation); br_f = cast_f32(br).
     No integer multiplies anywhere (int TENSOR_SCALAR mult stalls ~7.5us
     while gpsimd gather-descriptor expansion is active).
  7. B_g (TensorE): two accumulating broadcast matmuls per batch:
     psD_g[128, 84b:84b+84] = 1*idx_f + 441*br_f, x4 along free via
     to_broadcast.  (base = idx + 441*(idx//7), patch-row units of 192.)
  8. X_g (DVE): idx16 slice = cast_i16(psD_g) + static table
     (112*(s%4) + 7*w + 3136*b), 16-wrapped, replicated x8 gpsimd cores.
  9. G_g (gpsimd SWDGE): 4 dma_gather calls (768/768/640/512 idxs) on
     queues 0-3 -> 4 core-pairs expand descriptors concurrently (~10ns/desc
     per pair, the pipeline's pacing resource).
 10. S_gc: per-call stores GT cols -> DRAM out on the sync HWDGE queue.
"""

import numpy as np

_N_CORES = 8
_B = 64
_B_LOC = _B // _N_CORES  # 8
_PATCH = 64
_GRID = 7
_NPATCH = 21
_PROW = _PATCH * 3            # 192 elements per patch-row
_OUT_ROWS_LOC = _B_LOC * _NPATCH  # 168
_NBLK = 4                     # blocks of 2 batches
_CALL_COLS = [6, 6, 5, 4]     # gather call sizes per block, in 128-row cols
_CALL_OFF = [0, 6, 12, 17]

_nc_cache = None


def build_program():
    """Build the per-core SPMD Bass program (cached)."""
    global _nc_cache
    if _nc_cache is not None:
        return _nc_cache

    import concourse.bass as bass
    import concourse.bacc as bacc
    import concourse.mybir as mybir

    f32 = mybir.dt.float32
    i16 = mybir.dt.int16
    i32 = mybir.dt.int32
    u32 = mybir.dt.uint32
    X = mybir.AxisListType.X
    Op = mybir.AluOpType
    Act = mybir.ActivationFunctionType

    nc = bacc.Bacc(num_swdge_queues=4)

    hm_in = nc.declare_dram_parameter(
        "heatmap", [_B_LOC, 448, 448, 1], f32, isOutput=False)
    img_in = nc.declare_dram_parameter(
        "image", [_B_LOC, 448, 448, 3], f32, isOutput=False)
    out_t = nc.declare_dram_parameter(
        "out", [_OUT_ROWS_LOC, _PATCH, _PATCH, 3], f32, isOutput=True)

    # Static part of the gather index list, int16 [16, 672]:
    # position i = R sits at [R%16, R//16]; R = 16*s + w;
    # static term = 7*(R%64) + 3136*(R//1344) = 112*(s%4) + 7*w + 3136*(s//84)
    s_ar = np.arange(672, dtype=np.int64)
    w_ar = np.arange(16, dtype=np.int64)
    st = (112 * (s_ar[None, :] % 4) + 7 * w_ar[:, None]
          + 3136 * (s_ar[None, :] // 84)).astype(np.int16)
    st = np.tile(st, (8, 1))  # replicate across the 8 gpsimd cores
    sttab_const = nc.inline_tensor(st, name="sttab_const")

    # heatmap batch view [8, 64, 7, 448]: partition rl = r%64, free (r//64, c)
    hm_bat = (hm_in[:]
              .rearrange("b (rhi rl) c one -> b rl rhi (c one)", rl=64))

    # image patch-row view: [25088, 192]
    img_rows = (img_in[:]
                .rearrange("b r c ch -> (b r c ch)")
                .rearrange("(n e) -> n e", e=_PROW))

    # output patch-row view [10752, 192] -> [p, c, e] with R = 128*c + p
    out_pc = (out_t[:]
              .rearrange("r a b c -> (r a b c)")
              .rearrange("(n e) -> n e", e=_PROW)
              .rearrange("(c p) e -> p c e", p=128))

    from contextlib import ExitStack

    with ExitStack() as ctx:
        e = ctx.enter_context
        hmB = [e(nc.sbuf_tensor(f"hm{g}", [128, 7, 448], f32))
               for g in range(_NBLK)]
        red = [e(nc.sbuf_tensor(f"red{g}", [128, 7, 7], f32))
               for g in range(_NBLK)]
        G64 = e(nc.sbuf_tensor("G64", [128, 2], f32))
        ones128 = e(nc.sbuf_tensor("ones128", [1, 128], f32))
        w441 = e(nc.sbuf_tensor("w441", [1, 128], f32))
        Vt = e(nc.sbuf_tensor("Vt", [2, 49], f32))
        vw0 = e(nc.sbuf_tensor("vw0", [2, 49], f32))
        vw1 = e(nc.sbuf_tensor("vw1", [2, 49], f32))
        m2 = e(nc.sbuf_tensor("m2", [2, 8], f32))
        idxu = [e(nc.sbuf_tensor(f"idxu{g}", [2, 24], u32))
                for g in range(_NBLK)]
        idx_f = [e(nc.sbuf_tensor(f"idx_f{g}", [2, _NPATCH], f32))
                 for g in range(_NBLK)]
        u_f = [e(nc.sbuf_tensor(f"u_f{g}", [2, _NPATCH], f32))
               for g in range(_NBLK)]
        br_i = [e(nc.sbuf_tensor(f"br_i{g}", [2, _NPATCH], i32))
                for g in range(_NBLK)]
        br_f = [e(nc.sbuf_tensor(f"br_f{g}", [2, _NPATCH], f32))
                for g in range(_NBLK)]
        flat84 = [e(nc.sbuf_tensor(f"flat84{g}", [1, 84], f32))
                  for g in range(_NBLK)]
        bias07 = e(nc.sbuf_tensor("bias07", [2, 1], f32))
        sttab = e(nc.sbuf_tensor("sttab", [128, 672], i16))
        idx16 = e(nc.sbuf_tensor("idx16", [128, 672], i16))
        warmidx = e(nc.sbuf_tensor("warmidx", [128, 8], i16))
        GT = e(nc.sbuf_tensor("GT", [128, 84, _PROW], f32))
        GTwarm = e(nc.sbuf_tensor("GTwarm", [128, 1, _PROW], f32))
        psV = [e(nc.psum_tensor(f"psV{g}", [2, 49], f32))
               for g in range(_NBLK)]
        psD = [e(nc.psum_tensor(f"psD{g}", [128, 168], f32))
               for g in range(_NBLK)]
        s_loadE = e(nc.semaphore("s_loadE"))
        s_loadO = e(nc.semaphore("s_loadO"))
        s_stt = e(nc.semaphore("s_stt"))
        s_red = e(nc.semaphore("s_red"))
        s_pmm = e(nc.semaphore("s_pmm"))
        s_topk = e(nc.semaphore("s_topk"))
        s_act = e(nc.semaphore("s_act"))
        s_brf = e(nc.semaphore("s_brf"))
        s_flat = e(nc.semaphore("s_flat"))
        s_bmm = e(nc.semaphore("s_bmm"))
        s_idx = e(nc.semaphore("s_idx"))
        s_ones = e(nc.semaphore("s_ones"))
        s_warm = e(nc.semaphore("s_warm"))
        s_gc = [e(nc.semaphore(f"s_gc{k}")) for k in range(4 * _NBLK)]
        s_st = e(nc.semaphore("s_st"))
        block = e(nc.Block())

        @block.sync
        def _(sync):
            for g in range(_NBLK):
                sync.dma_start(
                    out=hmB[g][0:64, :, :],
                    in_=hm_bat[2 * g],
                ).then_inc(s_loadE, 16)
            for g in range(_NBLK):
                for c in range(4):
                    k = 4 * g + c
                    lo = 21 * g + _CALL_OFF[c]
                    hi = lo + _CALL_COLS[c]
                    sync.wait_ge(s_gc[k], 16)
                    sync.dma_start(
                        out=out_pc[:, lo:hi, :],
                        in_=GT[:, lo:hi, :],
                    ).then_inc(s_st, 16)
            sync.wait_ge(s_st, 16 * 4 * _NBLK)

        @block.scalar
        def _(sc):
            sc.dma_start(out=sttab[:], in_=sttab_const[:]).then_inc(s_stt, 16)
            for g in range(_NBLK):
                sc.dma_start(
                    out=hmB[g][64:128, :, :],
                    in_=hm_bat[2 * g + 1],
                ).then_inc(s_loadO, 16)
            for g in range(_NBLK):
                # u = (idx + 0.5) / 7 on the Act engine (f32 only)
                sc.wait_ge(s_topk, g + 1)
                sc.activation(
                    out=u_f[g][:], in_=idx_f[g][:], func=Act.Identity,
                    scale=0.14285715, bias=bias07[:],
                )
                sc.drain().then_inc(s_act, 1)
                # flatten idx_f [2,21] -> [1,0:42] and br_f -> [1,42:84]
                sc.wait_ge(s_brf, g + 1)
                sc.dma_start(
                    out=flat84[g][:, 0:42], in_=idx_f[g][:],
                ).then_inc(s_flat, 16)
                sc.dma_start(
                    out=flat84[g][:, 42:84], in_=br_f[g][:],
                ).then_inc(s_flat, 16)

        @block.vector
        def _(vector):
            # constants (disjoint writes, no deps)
            vector.memset(G64[0:64, 0:1], 1.0)
            vector.memset(G64[0:64, 1:2], 0.0)
            vector.memset(G64[64:128, 0:1], 0.0)
            vector.memset(G64[64:128, 1:2], 1.0)
            vector.memset(ones128[:], 1.0)
            vector.memset(w441[:], 441.0)
            vector.memset(warmidx[:], 0)
            # DVE CAST f32->i32 rounds to nearest: floor(x) == round(x - 0.5)
            # u = (idx + 0.5)/7 - 0.5 keeps >=0.07 margin from half-integers
            vector.memset(bias07[:], 0.071428575 - 0.5)
            vector.drain().then_inc(s_ones, 1)

            def R_stage(g):
                vector.wait_ge(s_loadE, 16 * (g + 1))
                vector.wait_ge(s_loadO, 16 * (g + 1))
                vector.reduce_sum(
                    out=red[g][:],
                    in_=hmB[g][:].rearrange("p rhi (bc u) -> p rhi bc u",
                                            u=64),
                    axis=X,
                )
                vector.drain().then_inc(s_red, 1)

            def T_stage(g):
                vector.wait_ge(s_pmm, g + 1)
                vector.tensor_copy(out=Vt[:], in_=psV[g][:])
                vector.drain()
                cur = Vt
                for r3 in range(3):
                    vector.max(out=m2[:], in_=cur[:])
                    vector.drain()
                    vector.max_index(
                        out=idxu[g][:, 8 * r3:8 * r3 + 8], in_max=m2[:],
                        in_values=cur[:])
                    if r3 < 2:
                        nxt = vw0 if r3 == 0 else vw1
                        vector.match_replace(
                            out=nxt[:], in_to_replace=m2[:], in_values=cur[:],
                            imm_value=-1e30)
                        vector.drain()
                        cur = nxt
                vector.drain()
                vector.tensor_copy(out=idx_f[g][:], in_=idxu[g][:, :_NPATCH])
                vector.drain().then_inc(s_topk, 1)

            def C_stage(g):
                vector.wait_ge(s_act, g + 1)
                vector.tensor_copy(out=br_i[g][:], in_=u_f[g][:])
                vector.drain()
                vector.tensor_copy(out=br_f[g][:], in_=br_i[g][:])
                vector.drain().then_inc(s_brf, 1)

            def X_stage(g):
                vector.wait_ge(s_bmm, 2 * (g + 1))
                if g == 0:
                    vector.wait_ge(s_stt, 16)
                sl = slice(168 * g, 168 * g + 168)
                vector.tensor_copy(out=idx16[:, sl], in_=psD[g][:])
                vector.drain()
                vector.tensor_tensor(
                    out=idx16[:, sl], in0=idx16[:, sl], in1=sttab[:, sl],
                    op=Op.add)
                vector.drain().then_inc(s_idx, 1)

            R_stage(0)
            T_stage(0)
            C_stage(0)
            R_stage(1)
            X_stage(0)
            T_stage(1)
            C_stage(1)
            R_stage(2)
            X_stage(1)
            T_stage(2)
            C_stage(2)
            R_stage(3)
            X_stage(2)
            T_stage(3)
            C_stage(3)
            X_stage(3)

        @block.tensor
        def _(tensor):
            def P_stage(g):
                tensor.wait_ge(s_red, g + 1)
                if g == 0:
                    tensor.wait_ge(s_ones, 1)
                tensor.matmul(
                    out=psV[g][:],
                    lhsT=G64[:],
                    rhs=red[g][:].rearrange("p rhi bc -> p (rhi bc)"),
                    start=True,
                    stop=True,
                ).then_inc(s_pmm, 1)

            def B_stage(g):
                # psD[g] = idx + 441*br, x4 along free
                tensor.wait_ge(s_flat, 32 * (g + 1))
                ridx = (flat84[g][0:1, 0:42]
                        .rearrange("p (m one) -> p m one", one=1)
                        .to_broadcast([1, 42, 4]))
                rbr = (flat84[g][0:1, 42:84]
                       .rearrange("p (m one) -> p m one", one=1)
                       .to_broadcast([1, 42, 4]))
                tensor.matmul(
                    out=psD[g][:], lhsT=ones128[:], rhs=ridx,
                    start=True, stop=False)
                tensor.matmul(
                    out=psD[g][:], lhsT=w441[:], rhs=rbr,
                    start=False, stop=True,
                ).then_inc(s_bmm, 2)

            P_stage(0)
            P_stage(1)
            B_stage(0)
            P_stage(2)
            B_stage(1)
            P_stage(3)
            B_stage(2)
            B_stage(3)

        @block.gpsimd
        def _(g):
            # preload the extended-instruction library early so the ucode
            # overlay DMA overlaps the heatmap phase
            from concourse import library_config
            g.load_library(library_config.mlp)
            # dummy gather absorbs any one-time ucode init cost
            g.wait_ge(s_ones, 1)
            g.dma_gather(
                out_ap=GTwarm[:],
                in_ap=img_rows,
                idxs_ap=warmidx[:],
                num_idxs=128,
                num_idxs_reg=128,
                elem_size=_PROW,
                queue_num=0,
            ).then_inc(s_warm, 16)
            g.wait_ge(s_warm, 16)
            # real gathers: 4 calls per block on queues 0-3 so all four
            # SWDGE core-pairs expand descriptors concurrently
            for blk in range(_NBLK):
                g.wait_ge(s_idx, blk + 1)
                for c in range(4):
                    n = 128 * _CALL_COLS[c]
                    lo = 21 * blk + _CALL_OFF[c]
                    ilo = 168 * blk + 8 * _CALL_OFF[c]
                    g.dma_gather(
                        out_ap=GT[:, lo:lo + _CALL_COLS[c], :],
                        in_ap=img_rows,
                        idxs_ap=idx16[:, ilo:ilo + 8 * _CALL_COLS[c]],
                        num_idxs=n,
                        num_idxs_reg=n,
                        elem_size=_PROW,
                        queue_num=c,
                    ).then_inc(s_gc[4 * blk + c], 16)

    nc.finalize()
    _nc_cache = nc
    return nc


def kernel(heatmap, image):
    from concourse.bass_utils import run_bass_kernel_spmd

    heatmap = np.ascontiguousarray(np.asarray(heatmap), dtype=np.float32)
    image = np.ascontiguousarray(np.asarray(image), dtype=np.float32)
    assert heatmap.shape == (_B, 448, 448, 1)
    assert image.shape == (_B, 448, 448, 3)

    nc = build_program()
    in_maps = [
        {
            "heatmap": heatmap[c * _B_LOC:(c + 1) * _B_LOC],
            "image": image[c * _B_LOC:(c + 1) * _B_LOC],
        }
        for c in range(_N_CORES)
    ]
    res = run_bass_kernel_spmd(nc, in_maps, list(range(_N_CORES)))
    outs = [res.results[c]["out"] for c in range(_N_CORES)]
    return np.concatenate(outs, axis=0)


# revision 31
# speedup vs baseline: 1.3879x; 1.2212x over previous
"""Trainium2 Bass kernel for ExtractRelevantPatches (pool -> top-k -> gather).

Full-input contract: kernel(heatmap [64,448,448,1] f32, image [64,448,448,3] f32)
-> [1344, 64, 64, 3] f32.

Sharding: pure data-parallel over batch; 8 batches per NeuronCore, 8 cores.

Pipelined per-core algorithm (raw Bass, explicit semaphores), 4 blocks of
2 batches, so gather descriptor expansion + HBM flow of early blocks overlap
pool/top-k of later blocks:

  Per block g (batches 2g, 2g+1):
  1. L_g: heatmap batch 2g on the sync HWDGE queue -> SBUF partitions 0-63,
     batch 2g+1 on the scalar HWDGE queue -> partitions 64-127 (two queues
     so both 64-partition streams run concurrently; SBUF write bw is
     ~2.9 GB/s/partition).  Tile [128, 7, 448], partition = 64*bp + r%64.
  2. R_g (DVE): reduce_sum over 64-col groups -> red_g [128, 7, 7].
  3. P_g (TensorE): matmul with 0/1 matrix G64 (G64[p,b]=1 iff p//64==b)
     -> psV_g [2, 49]: per-batch pooled sums on their own partition.
  4. T_g (DVE): top-24 via 3 rounds of max8/find_index8/match_replace8,
     keep first 21 (descending, matching jax top_k); cast idx -> f32.
  5. A_g (Act): u = (idx + 0.5) * (1/7)  (scale+bias in one activation).
  6. C_g (DVE): br = cast_i32(u) (truncation); br_f = cast_f32(br).
     No integer multiplies anywhere (int TENSOR_SCALAR mult stalls ~7.5us
     while gpsimd gather-descriptor expansion is active).
  7. B_g (TensorE): two accumulating broadcast matmuls per batch:
     psD_g[128, 84b:84b+84] = 1*idx_f + 441*br_f, x4 along free via
     to_broadcast.  (base = idx + 441*(idx//7), patch-row units of 192.)
  8. X_g (DVE): idx16 slice = cast_i16(psD_g) + static table
     (112*(s%4) + 7*w + 3136*b), 16-wrapped, replicated x8 gpsimd cores.
  9. G_g (gpsimd SWDGE): 4 dma_gather calls (768/768/640/512 idxs) on
     queues 0-3 -> 4 core-pairs expand descriptors concurrently (~10ns/desc
     per pair, the pipeline's pacing resource).
 10. S_gc: per-call stores GT cols -> DRAM out on the sync HWDGE queue.
"""

import numpy as np

_N_CORES = 8
_B = 64
_B_LOC = _B // _N_CORES  # 8
_PATCH = 64
_GRID = 7
_NPATCH = 21
_PROW = _PATCH * 3            # 192 elements per patch-row
_OUT_ROWS_LOC = _B_LOC * _NPATCH  # 168
_NBLK = 4                     # blocks of 2 batches
_CALL_COLS = [6, 6, 5, 4]     # gather call sizes per block, in 128-row cols
_CALL_OFF = [0, 6, 12, 17]

_nc_cache = None


def build_program():
    """Build the per-core SPMD Bass program (cached)."""
    global _nc_cache
    if _nc_cache is not None:
        return _nc_cache

    import concourse.bass as bass
    import concourse.bacc as bacc
    import concourse.mybir as mybir

    f32 = mybir.dt.float32
    i16 = mybir.dt.int16
    i32 = mybir.dt.int32
    u32 = mybir.dt.uint32
    X = mybir.AxisListType.X
    Op = mybir.AluOpType
    Act = mybir.ActivationFunctionType

    nc = bacc.Bacc(num_swdge_queues=4)

    hm_in = nc.declare_dram_parameter(
        "heatmap", [_B_LOC, 448, 448, 1], f32, isOutput=False)
    img_in = nc.declare_dram_parameter(
        "image", [_B_LOC, 448, 448, 3], f32, isOutput=False)
    out_t = nc.declare_dram_parameter(
        "out", [_OUT_ROWS_LOC, _PATCH, _PATCH, 3], f32, isOutput=True)

    # Static parts of the gather index list: position i = R at [R%16, R//16],
    # R = 16*s + w; static term = 112*(s%4) + 7*w + 3136*(R//1344).
    # The s-dependent part rides a third accumulating matmul (statc row per
    # block); the w-dependent 7*(p%16) part rides the Act copy's bias.
    s_ar = np.arange(168, dtype=np.int64)
    stat = np.zeros((1, 672), dtype=np.float32)
    for g_ in range(4):
        stat[0, 168 * g_:168 * g_ + 168] = (
            112 * (s_ar % 4) + 3136 * (2 * g_ + s_ar // 84))
    statc_const = nc.inline_tensor(stat, name="statc_const")
    b7 = (7.0 * (np.arange(128) % 16)).reshape(128, 1).astype(np.float32)
    bias7w_const = nc.inline_tensor(b7, name="bias7w_const")

    # heatmap batch view [8, 64, 7, 448]: partition rl = r%64, free (r//64, c)
    hm_bat = (hm_in[:]
              .rearrange("b (rhi rl) c one -> b rl rhi (c one)", rl=64))

    # image patch-row view: [25088, 192]
    img_rows = (img_in[:]
                .rearrange("b r c ch -> (b r c ch)")
                .rearrange("(n e) -> n e", e=_PROW))

    # output patch-row view [10752, 192] -> [p, c, e] with R = 128*c + p
    out_pc = (out_t[:]
              .rearrange("r a b c -> (r a b c)")
              .rearrange("(n e) -> n e", e=_PROW)
              .rearrange("(c p) e -> p c e", p=128))

    from contextlib import ExitStack

    with ExitStack() as ctx:
        e = ctx.enter_context
        hmB = [e(nc.sbuf_tensor(f"hm{g}", [128, 7, 448], f32))
               for g in range(_NBLK)]
        red = [e(nc.sbuf_tensor(f"red{g}", [128, 7, 7], f32))
               for g in range(_NBLK)]
        G64 = e(nc.sbuf_tensor("G64", [128, 2], f32))
        ones128 = e(nc.sbuf_tensor("ones128", [1, 128], f32))
        w441 = e(nc.sbuf_tensor("w441", [1, 128], f32))
        Vt = e(nc.sbuf_tensor("Vt", [2, 49], f32))
        vw0 = e(nc.sbuf_tensor("vw0", [2, 49], f32))
        vw1 = e(nc.sbuf_tensor("vw1", [2, 49], f32))
        m2 = e(nc.sbuf_tensor("m2", [2, 8], f32))
        idxu = [e(nc.sbuf_tensor(f"idxu{g}", [2, 24], u32))
                for g in range(_NBLK)]
        idx_f = [e(nc.sbuf_tensor(f"idx_f{g}", [2, _NPATCH], f32))
                 for g in range(_NBLK)]
        u_f = [e(nc.sbuf_tensor(f"u_f{g}", [2, _NPATCH], f32))
               for g in range(_NBLK)]
        br_i = [e(nc.sbuf_tensor(f"br_i{g}", [2, _NPATCH], i32))
                for g in range(_NBLK)]
        br_f = [e(nc.sbuf_tensor(f"br_f{g}", [2, _NPATCH], f32))
                for g in range(_NBLK)]
        flat84 = [e(nc.sbuf_tensor(f"flat84{g}", [1, 84], f32))
                  for g in range(_NBLK)]
        bias07 = e(nc.sbuf_tensor("bias07", [2, 1], f32))
        statc = e(nc.sbuf_tensor("statc", [1, 672], f32))
        bias7w = e(nc.sbuf_tensor("bias7w", [128, 1], f32))
        idx16 = e(nc.sbuf_tensor("idx16", [128, 672], i16))
        warmidx = e(nc.sbuf_tensor("warmidx", [128, 8], i16))
        GT = e(nc.sbuf_tensor("GT", [128, 84, _PROW], f32))
        GTwarm = e(nc.sbuf_tensor("GTwarm", [128, 1, _PROW], f32))
        psV = [e(nc.psum_tensor(f"psV{g}", [2, 49], f32))
               for g in range(_NBLK)]
        psD = [e(nc.psum_tensor(f"psD{g}", [128, 168], f32))
               for g in range(_NBLK)]
        s_loadE = e(nc.semaphore("s_loadE"))
        s_loadO = e(nc.semaphore("s_loadO"))
        s_stt = e(nc.semaphore("s_stt"))
        s_red = e(nc.semaphore("s_red"))
        s_pmm = e(nc.semaphore("s_pmm"))
        s_topk = e(nc.semaphore("s_topk"))
        s_act = e(nc.semaphore("s_act"))
        s_brf = e(nc.semaphore("s_brf"))
        s_flat = e(nc.semaphore("s_flat"))
        s_psd = e(nc.semaphore("s_psd"))
        s_bmm = e(nc.semaphore("s_bmm"))
        s_ones = e(nc.semaphore("s_ones"))
        s_warm = e(nc.semaphore("s_warm"))
        s_gc = [e(nc.semaphore(f"s_gc{k}")) for k in range(4 * _NBLK)]
        s_st = e(nc.semaphore("s_st"))
        block = e(nc.Block())

        @block.sync
        def _(sync):
            for g in range(_NBLK):
                sync.dma_start(
                    out=hmB[g][0:64, :, :],
                    in_=hm_bat[2 * g],
                ).then_inc(s_loadE, 16)
            for g in range(_NBLK):
                for c in range(4):
                    k = 4 * g + c
                    lo = 21 * g + _CALL_OFF[c]
                    hi = lo + _CALL_COLS[c]
                    sync.wait_ge(s_gc[k], 16)
                    sync.dma_start(
                        out=out_pc[:, lo:hi, :],
                        in_=GT[:, lo:hi, :],
                    ).then_inc(s_st, 16)
            sync.wait_ge(s_st, 16 * 4 * _NBLK)

        @block.scalar
        def _(sc):
            sc.dma_start(
                out=hmB[0][64:128, :, :], in_=hm_bat[1],
            ).then_inc(s_loadO, 16)
            sc.dma_start(out=statc[:], in_=statc_const[:]).then_inc(s_stt, 16)
            sc.dma_start(
                out=bias7w[:], in_=bias7w_const[:]).then_inc(s_stt, 16)
            for g in range(1, _NBLK):
                sc.dma_start(
                    out=hmB[g][64:128, :, :],
                    in_=hm_bat[2 * g + 1],
                ).then_inc(s_loadO, 16)
            for g in range(_NBLK):
                # u = (idx + 0.5) / 7 on the Act engine (f32 only)
                sc.wait_ge(s_topk, g + 1)
                sc.activation(
                    out=u_f[g][:], in_=idx_f[g][:], func=Act.Identity,
                    scale=0.14285715, bias=bias07[:],
                )
                sc.drain().then_inc(s_act, 1)
                # flatten idx_f [2,21] -> [1,0:42] and br_f -> [1,42:84]
                sc.wait_ge(s_brf, g + 1)
                sc.dma_start(
                    out=flat84[g][:, 0:42], in_=idx_f[g][:],
                ).then_inc(s_flat, 16)
                sc.dma_start(
                    out=flat84[g][:, 42:84], in_=br_f[g][:],
                ).then_inc(s_flat, 16)
                # cast psD (PSUM, exact integers) + 7*(p%16) -> idx16 slice
                # (i16, SBUF); gpsimd cannot read PSUM, the Act engine can
                sc.wait_ge(s_bmm, g + 1)
                sl = slice(168 * g, 168 * g + 168)
                sc.activation(
                    out=idx16[:, sl], in_=psD[g][:], func=Act.Identity,
                    scale=1.0, bias=bias7w[:])
                sc.drain().then_inc(s_psd, 1)

        @block.vector
        def _(vector):
            # constants (disjoint writes, no deps)
            vector.memset(G64[0:64, 0:1], 1.0)
            vector.memset(G64[0:64, 1:2], 0.0)
            vector.memset(G64[64:128, 0:1], 0.0)
            vector.memset(G64[64:128, 1:2], 1.0)
            vector.memset(ones128[:], 1.0)
            vector.memset(w441[:], 441.0)
            vector.memset(warmidx[:], 0)
            # DVE CAST f32->i32 rounds to nearest: floor(x) == round(x - 0.5)
            # u = (idx + 0.5)/7 - 0.5 keeps >=0.07 margin from half-integers
            vector.memset(bias07[:], 0.071428575 - 0.5)
            vector.drain().then_inc(s_ones, 1)

            def R_stage(g):
                vector.wait_ge(s_loadE, 16 * (g + 1))
                vector.wait_ge(s_loadO, 16 * (g + 1))
                vector.reduce_sum(
                    out=red[g][:],
                    in_=hmB[g][:].rearrange("p rhi (bc u) -> p rhi bc u",
                                            u=64),
                    axis=X,
                )
                vector.drain().then_inc(s_red, 1)

            def T_stage(g):
                vector.wait_ge(s_pmm, g + 1)
                vector.tensor_copy(out=Vt[:], in_=psV[g][:])
                vector.drain()
                cur = Vt
                for r3 in range(3):
                    vector.max(out=m2[:], in_=cur[:])
                    vector.drain()
                    vector.max_index(
                        out=idxu[g][:, 8 * r3:8 * r3 + 8], in_max=m2[:],
                        in_values=cur[:])
                    if r3 < 2:
                        nxt = vw0 if r3 == 0 else vw1
                        vector.match_replace(
                            out=nxt[:], in_to_replace=m2[:], in_values=cur[:],
                            imm_value=-1e30)
                        vector.drain()
                        cur = nxt
                vector.drain()
                vector.tensor_copy(out=idx_f[g][:], in_=idxu[g][:, :_NPATCH])
                vector.drain().then_inc(s_topk, 1)

            def C_stage(g):
                vector.wait_ge(s_act, g + 1)
                vector.tensor_copy(out=br_i[g][:], in_=u_f[g][:])
                vector.drain()
                vector.tensor_copy(out=br_f[g][:], in_=br_i[g][:])
                vector.drain().then_inc(s_brf, 1)

            for g in range(_NBLK):
                R_stage(g)
                T_stage(g)
                C_stage(g)

        @block.tensor
        def _(tensor):
            def P_stage(g):
                tensor.wait_ge(s_red, g + 1)
                if g == 0:
                    tensor.wait_ge(s_ones, 1)
                tensor.matmul(
                    out=psV[g][:],
                    lhsT=G64[:],
                    rhs=red[g][:].rearrange("p rhi bc -> p (rhi bc)"),
                    start=True,
                    stop=True,
                ).then_inc(s_pmm, 1)

            def B_stage(g):
                # psD[g] = idx + 441*br (x4 along free) + static s-term
                tensor.wait_ge(s_flat, 32 * (g + 1))
                if g == 0:
                    tensor.wait_ge(s_stt, 32)
                ridx = (flat84[g][0:1, 0:42]
                        .rearrange("p (m one) -> p m one", one=1)
                        .to_broadcast([1, 42, 4]))
                rbr = (flat84[g][0:1, 42:84]
                       .rearrange("p (m one) -> p m one", one=1)
                       .to_broadcast([1, 42, 4]))
                tensor.matmul(
                    out=psD[g][:], lhsT=ones128[:], rhs=ridx,
                    start=True, stop=False)
                tensor.matmul(
                    out=psD[g][:], lhsT=w441[:], rhs=rbr,
                    start=False, stop=False)
                tensor.matmul(
                    out=psD[g][:], lhsT=ones128[:],
                    rhs=statc[0:1, 168 * g:168 * g + 168],
                    start=False, stop=True,
                ).then_inc(s_bmm, 1)

            P_stage(0)
            P_stage(1)
            B_stage(0)
            P_stage(2)
            B_stage(1)
            P_stage(3)
            B_stage(2)
            B_stage(3)

        @block.gpsimd
        def _(g):
            # preload the extended-instruction library early so the ucode
            # overlay DMA overlaps the heatmap phase
            from concourse import library_config
            g.load_library(library_config.mlp)
            # dummy gather absorbs any one-time ucode init cost
            g.wait_ge(s_ones, 1)
            g.dma_gather(
                out_ap=GTwarm[:],
                in_ap=img_rows,
                idxs_ap=warmidx[:],
                num_idxs=128,
                num_idxs_reg=128,
                elem_size=_PROW,
                queue_num=0,
            ).then_inc(s_warm, 16)
            g.wait_ge(s_warm, 16)
            # per block: build the index list (cast+add on gpsimd -- DVE
            # CAST/int ops stall ~5us when run concurrently with gather
            # descriptor expansion; on gpsimd they serialize cleanly),
            # then 4 gather calls on queues 0-3 so all four SWDGE
            # core-pairs expand descriptors concurrently
            for blk in range(_NBLK):
                g.wait_ge(s_psd, blk + 1)
                for c in range(4):
                    n = 128 * _CALL_COLS[c]
                    lo = 21 * blk + _CALL_OFF[c]
                    ilo = 168 * blk + 8 * _CALL_OFF[c]
                    g.dma_gather(
                        out_ap=GT[:, lo:lo + _CALL_COLS[c], :],
                        in_ap=img_rows,
                        idxs_ap=idx16[:, ilo:ilo + 8 * _CALL_COLS[c]],
                        num_idxs=n,
                        num_idxs_reg=n,
                        elem_size=_PROW,
                        queue_num=c,
                    ).then_inc(s_gc[4 * blk + c], 16)

    nc.finalize()
    _nc_cache = nc
    return nc


def kernel(heatmap, image):
    from concourse.bass_utils import run_bass_kernel_spmd

    heatmap = np.ascontiguousarray(np.asarray(heatmap), dtype=np.float32)
    image = np.ascontiguousarray(np.asarray(image), dtype=np.float32)
    assert heatmap.shape == (_B, 448, 448, 1)
    assert image.shape == (_B, 448, 448, 3)

    nc = build_program()
    in_maps = [
        {
            "heatmap": heatmap[c * _B_LOC:(c + 1) * _B_LOC],
            "image": image[c * _B_LOC:(c + 1) * _B_LOC],
        }
        for c in range(_N_CORES)
    ]
    res = run_bass_kernel_spmd(nc, in_maps, list(range(_N_CORES)))
    outs = [res.results[c]["out"] for c in range(_N_CORES)]
    return np.concatenate(outs, axis=0)


# revision 34
# speedup vs baseline: 1.3986x; 1.0077x over previous
"""Trainium2 Bass kernel for ExtractRelevantPatches (pool -> top-k -> gather).

Full-input contract: kernel(heatmap [64,448,448,1] f32, image [64,448,448,3] f32)
-> [1344, 64, 64, 3] f32.

Sharding: pure data-parallel over batch; 8 batches per NeuronCore, 8 cores.

Pipelined per-core algorithm (raw Bass, explicit semaphores), 4 blocks of
2 batches, so gather descriptor expansion + HBM flow of early blocks overlap
pool/top-k of later blocks:

  Per block g (batches 2g, 2g+1):
  1. L_g: heatmap batch 2g on the sync HWDGE queue -> SBUF partitions 0-63,
     batch 2g+1 on the scalar HWDGE queue -> partitions 64-127 (two queues
     so both 64-partition streams run concurrently; SBUF write bw is
     ~2.9 GB/s/partition).  Tile [128, 7, 448], partition = 64*bp + r%64.
  2. R_g (DVE): reduce_sum over 64-col groups -> red_g [128, 7, 7].
  3. P_g (TensorE): matmul with 0/1 matrix G64 (G64[p,b]=1 iff p//64==b)
     -> psV_g [2, 49]: per-batch pooled sums on their own partition.
  4. T_g (DVE): top-24 via 3 rounds of max8/find_index8/match_replace8,
     keep first 21 (descending, matching jax top_k); cast idx -> f32.
  5. A_g (Act): u = (idx + 0.5) * (1/7)  (scale+bias in one activation).
  6. C_g (DVE): br = cast_i32(u) (truncation); br_f = cast_f32(br).
     No integer multiplies anywhere (int TENSOR_SCALAR mult stalls ~7.5us
     while gpsimd gather-descriptor expansion is active).
  7. B_g (TensorE): two accumulating broadcast matmuls per batch:
     psD_g[128, 84b:84b+84] = 1*idx_f + 441*br_f, x4 along free via
     to_broadcast.  (base = idx + 441*(idx//7), patch-row units of 192.)
  8. X_g (DVE): idx16 slice = cast_i16(psD_g) + static table
     (112*(s%4) + 7*w + 3136*b), 16-wrapped, replicated x8 gpsimd cores.
  9. G_g (gpsimd SWDGE): 4 dma_gather calls (768/768/640/512 idxs) on
     queues 0-3 -> 4 core-pairs expand descriptors concurrently (~10ns/desc
     per pair, the pipeline's pacing resource).
 10. S_gc: per-call stores GT cols -> DRAM out on the sync HWDGE queue.
"""

import numpy as np

_N_CORES = 8
_B = 64
_B_LOC = _B // _N_CORES  # 8
_PATCH = 64
_GRID = 7
_NPATCH = 21
_PROW = _PATCH * 3            # 192 elements per patch-row
_OUT_ROWS_LOC = _B_LOC * _NPATCH  # 168
_NBLK = 4                     # blocks of 2 batches
_CALL_COLS = [6, 6, 5, 4]     # gather call sizes per block, in 128-row cols
_CALL_OFF = [0, 6, 12, 17]

_nc_cache = None


def build_program():
    """Build the per-core SPMD Bass program (cached)."""
    global _nc_cache
    if _nc_cache is not None:
        return _nc_cache

    import concourse.bass as bass
    import concourse.bacc as bacc
    import concourse.mybir as mybir

    f32 = mybir.dt.float32
    i16 = mybir.dt.int16
    i32 = mybir.dt.int32
    u32 = mybir.dt.uint32
    X = mybir.AxisListType.X
    Op = mybir.AluOpType
    Act = mybir.ActivationFunctionType

    nc = bacc.Bacc(num_swdge_queues=4)

    hm_in = nc.declare_dram_parameter(
        "heatmap", [_B_LOC, 448, 448, 1], f32, isOutput=False)
    img_in = nc.declare_dram_parameter(
        "image", [_B_LOC, 448, 448, 3], f32, isOutput=False)
    out_t = nc.declare_dram_parameter(
        "out", [_OUT_ROWS_LOC, _PATCH, _PATCH, 3], f32, isOutput=True)

    # Static parts of the gather index list: position i = R at [R%16, R//16],
    # R = 16*s + w; static term = 112*(s%4) + 7*w + 3136*(R//1344).
    # The s-dependent part rides a third accumulating matmul (statc row per
    # block); the w-dependent 7*(p%16) part rides the Act copy's bias.
    s_ar = np.arange(168, dtype=np.int64)
    stat = np.zeros((1, 672), dtype=np.float32)
    for g_ in range(4):
        stat[0, 168 * g_:168 * g_ + 168] = (
            112 * (s_ar % 4) + 3136 * (2 * g_ + s_ar // 84))
    statc_const = nc.inline_tensor(stat, name="statc_const")
    b7 = (7.0 * (np.arange(128) % 16)).reshape(128, 1).astype(np.float32)
    bias7w_const = nc.inline_tensor(b7, name="bias7w_const")

    # heatmap batch view [8, 64, 7, 448]: partition rl = r%64, free (r//64, c)
    hm_bat = (hm_in[:]
              .rearrange("b (rhi rl) c one -> b rl rhi (c one)", rl=64))

    # image patch-row view: [25088, 192]
    img_rows = (img_in[:]
                .rearrange("b r c ch -> (b r c ch)")
                .rearrange("(n e) -> n e", e=_PROW))

    # output patch-row view [10752, 192] -> [p, c, e] with R = 128*c + p
    out_pc = (out_t[:]
              .rearrange("r a b c -> (r a b c)")
              .rearrange("(n e) -> n e", e=_PROW)
              .rearrange("(c p) e -> p c e", p=128))

    from contextlib import ExitStack

    with ExitStack() as ctx:
        e = ctx.enter_context
        hmB = [e(nc.sbuf_tensor(f"hm{g}", [128, 7, 448], f32))
               for g in range(_NBLK)]
        red = [e(nc.sbuf_tensor(f"red{g}", [128, 7, 7], f32))
               for g in range(_NBLK)]
        G64 = e(nc.sbuf_tensor("G64", [128, 2], f32))
        ones128 = e(nc.sbuf_tensor("ones128", [1, 128], f32))
        w441 = e(nc.sbuf_tensor("w441", [1, 128], f32))
        Vt = e(nc.sbuf_tensor("Vt", [2, 49], f32))
        vw0 = e(nc.sbuf_tensor("vw0", [2, 49], f32))
        vw1 = e(nc.sbuf_tensor("vw1", [2, 49], f32))
        m2 = e(nc.sbuf_tensor("m2", [2, 8], f32))
        idxu = [e(nc.sbuf_tensor(f"idxu{g}", [2, 24], u32))
                for g in range(_NBLK)]
        idx_f = [e(nc.sbuf_tensor(f"idx_f{g}", [2, _NPATCH], f32))
                 for g in range(_NBLK)]
        u_f = [e(nc.sbuf_tensor(f"u_f{g}", [2, _NPATCH], f32))
               for g in range(_NBLK)]
        br_i = [e(nc.sbuf_tensor(f"br_i{g}", [2, _NPATCH], i32))
                for g in range(_NBLK)]
        br_f = [e(nc.sbuf_tensor(f"br_f{g}", [2, _NPATCH], f32))
                for g in range(_NBLK)]
        bias07 = e(nc.sbuf_tensor("bias07", [2, 1], f32))
        flat84 = [e(nc.sbuf_tensor(f"flat84{g}", [1, 84], f32))
                  for g in range(_NBLK)]
        statc = e(nc.sbuf_tensor("statc", [1, 672], f32))
        bias7w = e(nc.sbuf_tensor("bias7w", [128, 1], f32))
        idx16 = e(nc.sbuf_tensor("idx16", [128, 672], i16))
        warmidx = e(nc.sbuf_tensor("warmidx", [128, 8], i16))
        GT = e(nc.sbuf_tensor("GT", [128, 84, _PROW], f32))
        GTwarm = e(nc.sbuf_tensor("GTwarm", [128, 1, _PROW], f32))
        psV = [e(nc.psum_tensor(f"psV{g}", [2, 49], f32))
               for g in range(_NBLK)]
        psD = [e(nc.psum_tensor(f"psD{g}", [128, 168], f32))
               for g in range(_NBLK)]
        s_loadE = e(nc.semaphore("s_loadE"))
        s_loadO = e(nc.semaphore("s_loadO"))
        s_stt = e(nc.semaphore("s_stt"))
        s_red = e(nc.semaphore("s_red"))
        s_pmm = e(nc.semaphore("s_pmm"))
        s_topk = e(nc.semaphore("s_topk"))
        s_act = e(nc.semaphore("s_act"))
        s_brf = e(nc.semaphore("s_brf"))
        s_psd = e(nc.semaphore("s_psd"))
        s_flat = e(nc.semaphore("s_flat"))
        s_bmm = e(nc.semaphore("s_bmm"))
        s_ones = e(nc.semaphore("s_ones"))
        s_warm = e(nc.semaphore("s_warm"))
        s_gc = [e(nc.semaphore(f"s_gc{k}")) for k in range(4 * _NBLK)]
        s_st = e(nc.semaphore("s_st"))
        block = e(nc.Block())

        @block.sync
        def _(sync):
            for g in range(_NBLK):
                sync.dma_start(
                    out=hmB[g][0:64, :, :],
                    in_=hm_bat[2 * g],
                ).then_inc(s_loadE, 16)
            def flat(g):
                # flatten [2,21] -> [1,42] halves of flat84 (cross-partition
                # move; on the sync queue, which is idle here, so its
                # in-flight latency stays ~1us)
                sync.wait_ge(s_brf, g + 1)
                sync.dma_start(
                    out=flat84[g][:, 0:42], in_=idx_f[g][:],
                ).then_inc(s_flat, 16)
                sync.dma_start(
                    out=flat84[g][:, 42:84], in_=br_f[g][:],
                ).then_inc(s_flat, 16)

            def stores(g):
                for c in range(4):
                    k = 4 * g + c
                    lo = 21 * g + _CALL_OFF[c]
                    hi = lo + _CALL_COLS[c]
                    sync.wait_ge(s_gc[k], 16)
                    sync.dma_start(
                        out=out_pc[:, lo:hi, :],
                        in_=GT[:, lo:hi, :],
                    ).then_inc(s_st, 16)

            flat(0)
            flat(1)
            flat(2)
            stores(0)
            flat(3)
            stores(1)
            stores(2)
            stores(3)
            sync.wait_ge(s_st, 16 * 4 * _NBLK)

        @block.scalar
        def _(sc):
            sc.dma_start(
                out=hmB[0][64:128, :, :], in_=hm_bat[1],
            ).then_inc(s_loadO, 16)
            sc.dma_start(out=statc[:], in_=statc_const[:]).then_inc(s_stt, 16)
            sc.dma_start(
                out=bias7w[:], in_=bias7w_const[:]).then_inc(s_stt, 16)
            for g in range(1, _NBLK):
                sc.dma_start(
                    out=hmB[g][64:128, :, :],
                    in_=hm_bat[2 * g + 1],
                ).then_inc(s_loadO, 16)
            for g in range(_NBLK):
                # u = (idx + 0.5) / 7 on the Act engine (f32 only)
                sc.wait_ge(s_topk, g + 1)
                sc.activation(
                    out=u_f[g][:], in_=idx_f[g][:], func=Act.Identity,
                    scale=0.14285715, bias=bias07[:],
                )
                sc.drain().then_inc(s_act, 1)
                # cast psD (PSUM, exact integers) + 7*(p%16) -> idx16 slice
                # (i16, SBUF); gpsimd cannot read PSUM, the Act engine can
                sc.wait_ge(s_bmm, g + 1)
                sl = slice(168 * g, 168 * g + 168)
                sc.activation(
                    out=idx16[:, sl], in_=psD[g][:], func=Act.Identity,
                    scale=1.0, bias=bias7w[:])
                sc.drain().then_inc(s_psd, 1)

        @block.vector
        def _(vector):
            # constants (disjoint writes, no deps)
            vector.memset(G64[0:64, 0:1], 1.0)
            vector.memset(G64[0:64, 1:2], 0.0)
            vector.memset(G64[64:128, 0:1], 0.0)
            vector.memset(G64[64:128, 1:2], 1.0)
            vector.memset(ones128[:], 1.0)
            vector.memset(w441[:], 441.0)
            vector.memset(warmidx[:], 0)
            # DVE CAST f32->i32 rounds to nearest: floor(x) == round(x - 0.5)
            # u = (idx + 0.5)/7 - 0.5 keeps >=0.07 margin from half-integers
            vector.memset(bias07[:], 0.071428575 - 0.5)
            vector.drain().then_inc(s_ones, 1)

            def R_stage(g):
                vector.wait_ge(s_loadE, 16 * (g + 1))
                vector.wait_ge(s_loadO, 16 * (g + 1))
                vector.reduce_sum(
                    out=red[g][:],
                    in_=hmB[g][:].rearrange("p rhi (bc u) -> p rhi bc u",
                                            u=64),
                    axis=X,
                )
                vector.drain().then_inc(s_red, 1)

            def T_stage(g):
                vector.wait_ge(s_pmm, g + 1)
                vector.tensor_copy(out=Vt[:], in_=psV[g][:])
                vector.drain()
                cur = Vt
                for r3 in range(3):
                    vector.max(out=m2[:], in_=cur[:])
                    vector.drain()
                    vector.max_index(
                        out=idxu[g][:, 8 * r3:8 * r3 + 8], in_max=m2[:],
                        in_values=cur[:])
                    if r3 < 2:
                        nxt = vw0 if r3 == 0 else vw1
                        vector.match_replace(
                            out=nxt[:], in_to_replace=m2[:], in_values=cur[:],
                            imm_value=-1e30)
                        vector.drain()
                        cur = nxt
                vector.drain()
                vector.tensor_copy(out=idx_f[g][:], in_=idxu[g][:, :_NPATCH])
                vector.drain().then_inc(s_topk, 1)

            def C_stage(g):
                vector.wait_ge(s_act, g + 1)
                vector.tensor_copy(out=br_i[g][:], in_=u_f[g][:])
                vector.drain()
                vector.tensor_copy(out=br_f[g][:], in_=br_i[g][:])
                vector.drain().then_inc(s_brf, 1)

            for g in range(_NBLK):
                R_stage(g)
                T_stage(g)
                C_stage(g)

        @block.tensor
        def _(tensor):
            def P_stage(g):
                tensor.wait_ge(s_red, g + 1)
                if g == 0:
                    tensor.wait_ge(s_ones, 1)
                tensor.matmul(
                    out=psV[g][:],
                    lhsT=G64[:],
                    rhs=red[g][:].rearrange("p rhi bc -> p (rhi bc)"),
                    start=True,
                    stop=True,
                ).then_inc(s_pmm, 1)

            def B_stage(g):
                # psD[g] = idx + 441*br (x4 along free) + static s-term
                tensor.wait_ge(s_flat, 32 * (g + 1))
                if g == 0:
                    tensor.wait_ge(s_stt, 32)
                ridx = (flat84[g][0:1, 0:42]
                        .rearrange("p (m one) -> p m one", one=1)
                        .to_broadcast([1, 42, 4]))
                rbr = (flat84[g][0:1, 42:84]
                       .rearrange("p (m one) -> p m one", one=1)
                       .to_broadcast([1, 42, 4]))
                tensor.matmul(
                    out=psD[g][:], lhsT=ones128[:], rhs=ridx,
                    start=True, stop=False)
                tensor.matmul(
                    out=psD[g][:], lhsT=w441[:], rhs=rbr,
                    start=False, stop=False)
                tensor.matmul(
                    out=psD[g][:], lhsT=ones128[:],
                    rhs=statc[0:1, 168 * g:168 * g + 168],
                    start=False, stop=True,
                ).then_inc(s_bmm, 1)

            P_stage(0)
            P_stage(1)
            B_stage(0)
            P_stage(2)
            B_stage(1)
            P_stage(3)
            B_stage(2)
            B_stage(3)

        @block.gpsimd
        def _(g):
            # preload the extended-instruction library early so the ucode
            # overlay DMA overlaps the heatmap phase
            from concourse import library_config
            g.load_library(library_config.mlp)
            # dummy gather absorbs any one-time ucode init cost
            g.wait_ge(s_ones, 1)
            g.dma_gather(
                out_ap=GTwarm[:],
                in_ap=img_rows,
                idxs_ap=warmidx[:],
                num_idxs=128,
                num_idxs_reg=128,
                elem_size=_PROW,
                queue_num=0,
            ).then_inc(s_warm, 16)
            g.wait_ge(s_warm, 16)
            # per block: build the index list (cast+add on gpsimd -- DVE
            # CAST/int ops stall ~5us when run concurrently with gather
            # descriptor expansion; on gpsimd they serialize cleanly),
            # then 4 gather calls on queues 0-3 so all four SWDGE
            # core-pairs expand descriptors concurrently
            for blk in range(_NBLK):
                g.wait_ge(s_psd, blk + 1)
                for c in range(4):
                    n = 128 * _CALL_COLS[c]
                    lo = 21 * blk + _CALL_OFF[c]
                    ilo = 168 * blk + 8 * _CALL_OFF[c]
                    g.dma_gather(
                        out_ap=GT[:, lo:lo + _CALL_COLS[c], :],
                        in_ap=img_rows,
                        idxs_ap=idx16[:, ilo:ilo + 8 * _CALL_COLS[c]],
                        num_idxs=n,
                        num_idxs_reg=n,
                        elem_size=_PROW,
                        queue_num=c,
                    ).then_inc(s_gc[4 * blk + c], 16)

    nc.finalize()
    _nc_cache = nc
    return nc


def kernel(heatmap, image):
    from concourse.bass_utils import run_bass_kernel_spmd

    heatmap = np.ascontiguousarray(np.asarray(heatmap), dtype=np.float32)
    image = np.ascontiguousarray(np.asarray(image), dtype=np.float32)
    assert heatmap.shape == (_B, 448, 448, 1)
    assert image.shape == (_B, 448, 448, 3)

    nc = build_program()
    in_maps = [
        {
            "heatmap": heatmap[c * _B_LOC:(c + 1) * _B_LOC],
            "image": image[c * _B_LOC:(c + 1) * _B_LOC],
        }
        for c in range(_N_CORES)
    ]
    res = run_bass_kernel_spmd(nc, in_maps, list(range(_N_CORES)))
    outs = [res.results[c]["out"] for c in range(_N_CORES)]
    return np.concatenate(outs, axis=0)
